# revision 2
# baseline (speedup 1.0000x reference)
"""Block-causal self-attention TRN2 kernel, v2.

Sharding: 64 (batch x block) units -> 8 per NeuronCore, zero cross-core
traffic. Full inputs in, full output out.

v2 changes vs baseline (929us/core -> target <450us):
- fp16 for x^T / w_qkv / qk tiles (FWL 2x weight loads), bf16 for
  er / v / yT / w_proj paths. PSUM stays fp32.
- softmax denominators accumulated into per-2-pair [4, 256] PSUM tiles
  via one-hot lhsT matmuls; ONE reciprocal_approx_fast per group
  (replaces 64 single-partition RECIPROCALs = 242us DVE).
- rmsnorm rsqrt via ACT ln + exp(-0.5 x) (same table set as the softmax
  exp -> no table thrash; replaces 16 more slow RECIPROCALs + Sqrt set).
- causal dead quadrant (tk=1, tq<128) never computed; masking only on
  the two diagonal 128-blocks, via gpsimd affine_select (off DVE).
- av matmuls col-packed 2 heads (tile_position) -> yT [128, 256] tiles
  directly; no [64,...] repacking DMA.
- 4 transposes batched per PSUM bank, single-CAST evacuations.
- PSUM: front2(2) + qkb(2) + small(2) + attn(2) = 8 banks, front/attn
  disjoint so consecutive blocks overlap on PE.
"""

import numpy as np

import concourse.bacc as bacc
import concourse.tile as tile
from concourse import mybir
from concourse.bass_utils import run_bass_kernel_spmd

F32 = mybir.dt.float32
F32R = mybir.dt.float32r
F16 = mybir.dt.float16
BF16 = mybir.dt.bfloat16

B, T, C = 4, 4096, 1024
H, HD, LS = 16, 64, 256
NCORES = 8
NBLK = (B * T) // LS  # 64
BPC = NBLK // NCORES  # 8 blocks per core
TOK = BPC * LS  # 2048 tokens per core
CT = C // 128  # 8 c-tiles
EPS = 1e-6

AF = mybir.ActivationFunctionType
OP = mybir.AluOpType


def build(variant=8):
    nc = bacc.Bacc()
    x = nc.declare_dram_parameter("x", [TOK, C], F32, isOutput=False)
    w_qkv = nc.declare_dram_parameter("w_qkv", [C, 3 * C], F32, isOutput=False)
    ln_w = nc.declare_dram_parameter("ln_w", [C], F32, isOutput=False)
    w_proj = nc.declare_dram_parameter("w_proj", [C, C], F32, isOutput=False)
    out = nc.declare_dram_parameter("out", [TOK, C], F32, isOutput=True)

    with tile.TileContext(nc) as tc:
        with (
            tc.tile_pool(name="const", bufs=1) as cpool,
            tc.tile_pool(name="wstage", bufs=2) as wstage_pool,
            tc.tile_pool(name="xn", bufs=3) as xn_pool,
            tc.tile_pool(name="xt", bufs=2) as xt_pool,
            tc.tile_pool(name="qk", bufs=2) as qk_pool,
            tc.tile_pool(name="work", bufs=2) as work,
            tc.tile_pool(name="er", bufs=3) as er_pool,
            tc.tile_pool(name="yt", bufs=2) as yt_pool,
            tc.tile_pool(name="small", bufs=2) as small,
            tc.tile_pool(name="psF", bufs=3, space="PSUM") as psF,   # front
            tc.tile_pool(name="psS", bufs=2, space="PSUM") as psS,   # small sums
            tc.tile_pool(name="psA", bufs=3, space="PSUM") as psA,   # attn+proj
        ):
            # ---------------- constants ----------------
            # w_qkv as fp16 [128, ct, 3C]; staged f32 chunks, cast on ACT/DVE
            wq_sb = cpool.tile([128, CT, 3 * C], F16)
            for ct in range(CT):
                ws = wstage_pool.tile([128, 3 * C], F32, tag="wstage", name=f"ws{ct}")
                nc.sync.dma_start(out=ws, in_=w_qkv[ct * 128 : (ct + 1) * 128, :])
                if ct % 2 == 0:
                    nc.scalar.activation(out=wq_sb[:, ct], in_=ws, func=AF.Copy)
                else:
                    nc.vector.tensor_copy(out=wq_sb[:, ct], in_=ws)
            # w_proj as bf16 [128, ct, C]
            wp_sb = cpool.tile([128, CT, C], BF16)
            for ct in range(CT):
                ws = wstage_pool.tile([128, C], F32, tag="wpstage", name=f"wps{ct}")
                nc.sync.dma_start(out=ws, in_=w_proj[ct * 128 : (ct + 1) * 128, :])
                if ct % 2 == 0:
                    nc.scalar.activation(out=wp_sb[:, ct], in_=ws, func=AF.Copy)
                else:
                    nc.vector.tensor_copy(out=wp_sb[:, ct], in_=ws)
            # ln_w scaled: lnq = ln*4 (folds rmsnorm 32/sqrt + 1/8 head scale),
            # lnk = ln*32
            ln_f32 = cpool.tile([128, CT], F32)
            nc.sync.dma_start(out=ln_f32, in_=ln_w.rearrange("(ct p) -> p ct", p=128))
            lnq = cpool.tile([128, CT], F32)
            nc.vector.tensor_scalar_mul(lnq, ln_f32, 4.0)
            lnk = cpool.tile([128, CT], F32)
            nc.vector.tensor_scalar_mul(lnk, ln_f32, 32.0)

            ones_stage = cpool.tile([128, 128], F32)
            nc.vector.memset(ones_stage, 1.0)
            ones_row = cpool.tile([1, 128], F32R)
            nc.vector.tensor_copy(out=ones_row, in_=ones_stage[0:1, :])
            # identity for PE transpose
            ident_stage = cpool.tile([128, 128], F32)
            nc.vector.memset(ident_stage, 1.0)
            nc.gpsimd.affine_select(
                out=ident_stage,
                in_=ident_stage,
                pattern=[[1, 128]],
                compare_op=OP.is_equal,
                fill=0.0,
                base=0,
                channel_multiplier=-1,
            )
            ident = cpool.tile([128, 128], F32R)
            nc.vector.tensor_copy(out=ident, in_=ident_stage)
            # selector tiles (only full-partition writes / column slices are
            # legal: engine APs must start at 32-aligned partitions)
            # selq/selk [128, 2] fp16: one-hot column for q/k sumsq routing
            selq = cpool.tile([128, 2], F16)
            nc.vector.memset(selq, 0.0)
            nc.vector.memset(selq[:, 0:1], 1.0)
            selk = cpool.tile([128, 2], F16)
            nc.vector.memset(selk, 0.0)
            nc.vector.memset(selk[:, 1:2], 1.0)
            # selh[h2] [128, 2] bf16: one-hot column h2 (er sums routing)
            selh0 = cpool.tile([128, 2], BF16)
            nc.vector.memset(selh0, 0.0)
            nc.vector.memset(selh0[:, 0:1], 1.0)
            selh1 = cpool.tile([128, 2], BF16)
            nc.vector.memset(selh1, 0.0)
            nc.vector.memset(selh1[:, 1:2], 1.0)
            # onesq2/onesk2 [2, 128]: all-ones row 0 / row 1 (rank-1 broadcast
            # of one row of a [2, N] rhs). Staged in f32, rounded to f32r by
            # the final copy (verifier requires f32r matmul inputs be
            # produced "rounded").
            sel_stage = cpool.tile([2, 128], F32)
            nc.vector.memset(sel_stage, 1.0)
            nc.gpsimd.affine_select(
                out=sel_stage, in_=sel_stage, pattern=[[0, 128]],
                compare_op=OP.is_equal, fill=0.0, base=0, channel_multiplier=-1,
            )
            onesq2 = cpool.tile([2, 128], F32R)
            nc.vector.tensor_copy(out=onesq2, in_=sel_stage)
            sel_stage2 = cpool.tile([2, 128], F32)
            nc.vector.memset(sel_stage2, 1.0)
            nc.gpsimd.affine_select(
                out=sel_stage2, in_=sel_stage2, pattern=[[0, 128]],
                compare_op=OP.is_equal, fill=0.0, base=-1, channel_multiplier=1,
            )
            onesk2 = cpool.tile([2, 128], F32R)
            nc.vector.tensor_copy(out=onesk2, in_=sel_stage2)
            # sel2 [2, 128] bf16: row0 one for cols 0-63, row1 one for 64-127
            sel_stage3 = cpool.tile([2, 128], F32)
            nc.vector.memset(sel_stage3, 1.0)
            nc.gpsimd.affine_select(
                out=sel_stage3, in_=sel_stage3, pattern=[[1, 128]],
                compare_op=OP.is_ge, fill=0.0, base=0, channel_multiplier=-64,
            )
            nc.gpsimd.affine_select(
                out=sel_stage3, in_=sel_stage3, pattern=[[-1, 128]],
                compare_op=OP.is_ge, fill=0.0, base=63, channel_multiplier=64,
            )
            sel2 = cpool.tile([2, 128], F32R)
            nc.vector.tensor_copy(out=sel2, in_=sel_stage3)
            # rmsnorm ln bias: sumsq + 1024*eps
            bias_eps = cpool.tile([2, 1], F32)
            nc.vector.memset(bias_eps, float(1024.0 * EPS))

            for b in range(BPC):
                t0 = b * LS
                # ---- natural x load + PE transpose (4 per bank) -> xT fp16 ----
                xT = xt_pool.tile([128, CT, LS], F16, tag="xT", name=f"xT_{b}")
                for tt in range(2):
                    x_nat = xn_pool.tile(
                        [128, C], F32R, tag="xn", name=f"xn_{b}_{tt}"
                    )
                    nc.scalar.dma_start(
                        out=x_nat,
                        in_=x[t0 + tt * 128 : t0 + (tt + 1) * 128, :].bitcast(F32R),
                    )
                    for g in range(2):  # 4 transposes per bank
                        tp = psF.tile(
                            [128, 512], F32R, tag="psF", name=f"tp_{b}_{tt}_{g}"
                        )
                        for q in range(4):
                            ct = g * 4 + q
                            nc.tensor.transpose(
                                tp[:, q * 128 : (q + 1) * 128],
                                x_nat[:, ct * 128 : (ct + 1) * 128],
                                ident,
                            )
                        nc.vector.tensor_copy(
                            out=xT[:, g * 4 : (g + 1) * 4, tt * 128 : (tt + 1) * 128],
                            in_=tp.bitcast(F32).rearrange("p (c t) -> p c t", t=128),
                        )

                # ---- qk d-tiles (0..7 q, 8..15 k), 2 dts per bank ----
                qk_sb = qk_pool.tile([128, 16, LS], F32R, tag="qk", name=f"qk_{b}")
                smq = psS.tile([2, LS], F32, tag="psS", name=f"smq_{b}")
                for g in range(8):  # dt pair (2g, 2g+1)
                    ps = psF.tile([128, 512], F32, tag="psF", name=f"qkps_{b}_{g}")
                    for half in range(2):
                        dt = 2 * g + half
                        for ct in range(CT):
                            nc.tensor.matmul(
                                ps[:, half * LS : (half + 1) * LS],
                                wq_sb[:, ct, dt * 128 : (dt + 1) * 128],
                                xT[:, ct, :],
                                start=(ct == 0),
                                stop=(ct == CT - 1),
                                skip_group_check=True,
                            )
                    # squares (ACT) -> q2 fp16, then one-hot sum matmuls
                    q2 = work.tile([128, 512], F16, tag="q2", name=f"q2_{b}_{g}")
                    nc.scalar.activation(out=q2, in_=ps, func=AF.Square)
                    for half in range(2):
                        dt = 2 * g + half
                        sel = selq if dt < 8 else selk
                        nc.tensor.matmul(
                            smq,
                            sel,
                            q2[:, half * LS : (half + 1) * LS],
                            start=(dt == 0),
                            stop=(dt == 15),
                            skip_group_check=True,
                        )
                    # evacuate qk bank -> fp16
                    nc.vector.tensor_copy(
                        out=qk_sb[:, 2 * g : 2 * g + 2, :].rearrange(
                            "p d t -> p (d t)"
                        ),
                        in_=ps,
                    )

                # ---- v natural [tk, head*64] as bf16 ----
                v_sb = work.tile([128, 2, 2 * 512], BF16, tag="vsb", name=f"v_{b}")
                for tt in range(2):
                    for ch in range(2):
                        vps = psF.tile(
                            [128, 512], F32, tag="psF", name=f"vps_{b}_{tt}_{ch}"
                        )
                        for ct in range(CT):
                            nc.tensor.matmul(
                                vps,
                                xT[:, ct, tt * 128 : (tt + 1) * 128],
                                wq_sb[:, ct, 2 * C + ch * 512 : 2 * C + (ch + 1) * 512],
                                start=(ct == 0),
                                stop=(ct == CT - 1),
                            )
                        nc.scalar.activation(
                            out=v_sb[:, tt, ch * 512 : (ch + 1) * 512],
                            in_=vps,
                            func=AF.Copy,
                        )

                # ---- rmsnorm: r = exp(-0.5*ln(sumsq + 1024*eps)) ----
                lnt = small.tile([2, LS], F32, tag="lnt", name=f"lnt_{b}")
                nc.scalar.activation(out=lnt, in_=smq, func=AF.Ln, bias=bias_eps)
                r_f = small.tile([2, LS], F32, tag="rf", name=f"rf_{b}")
                nc.scalar.activation(out=r_f, in_=lnt, func=AF.Exp, scale=-0.5)
                r_ = small.tile([2, LS], F32R, tag="r", name=f"r_{b}")
                nc.vector.tensor_copy(out=r_, in_=r_f)
                rb_ps = psS.tile([128, 512], F32, tag="psS", name=f"rb_{b}")
                for half in range(2):
                    nc.tensor.matmul(
                        rb_ps[:, half * LS : (half + 1) * LS],
                        onesq2 if half == 0 else onesk2,
                        r_,
                        start=True,
                        stop=True,
                        skip_group_check=True,
                    )
                for dt in range(16):
                    half = dt // 8
                    ln_s = lnq if half == 0 else lnk
                    nc.vector.scalar_tensor_tensor(
                        out=qk_sb[:, dt, :],
                        in0=qk_sb[:, dt, :],
                        scalar=ln_s[:, dt % 8 : dt % 8 + 1],
                        in1=rb_ps[:, half * LS : (half + 1) * LS],
                        op0=OP.mult,
                        op1=OP.mult,
                    )

                if variant == 1:
                    dbg = work.tile([128, C], F32, tag="ostage", name=f"dbg_{b}")
                    nc.vector.tensor_copy(out=dbg[:, 0:512], in_=v_sb[:, 0, 0:512])
                    nc.vector.tensor_copy(
                        out=dbg[:, 512:768], in_=qk_sb[:, 0, :]
                    )
                    nc.vector.tensor_copy(
                        out=dbg[:, 768:1024], in_=qk_sb[:, 8, :]
                    )
                    nc.sync.dma_start(out=out[t0 : t0 + 128, :], in_=dbg)
                    o1 = work.tile([128, C], F32, tag="ostage", name=f"o1d_{b}")
                    nc.vector.memset(o1, 0.0)
                    nc.sync.dma_start(out=out[t0 + 128 : t0 + 256, :], in_=o1)
                    continue

                # ---- attention: 8 pairs (2 heads each), groups of 2 pairs ----
                yT = yt_pool.tile([128, 8, LS], BF16, tag="yT", name=f"yT_{b}")
                dbg_consume = []
                for j in range(8):
                    kx = qk_sb[:, 8 + j, :]
                    qx = qk_sb[:, j, :]
                    # scores tk0: keys 0-127, all queries. One PSUM
                    # tile + one exp per h2 (baseline-proven pattern)
                    er0 = er_pool.tile(
                        [128, 512], BF16, tag="er0", name=f"er0_{b}_{j}"
                    )
                    for h2 in range(2):
                        po = 64 * h2
                        sc0 = psA.tile(
                            [128, LS], F32, tag="psA", name=f"sc0_{b}_{j}_{h2}"
                        )
                        nc.tensor.matmul(
                            sc0,
                            kx[po : po + 64, 0:128],
                            qx[po : po + 64, :],
                            start=True,
                            stop=True,
                        )
                        nc.scalar.activation(
                            out=er0[:, h2 * LS : (h2 + 1) * LS],
                            in_=sc0,
                            func=AF.Exp,
                        )
                    # scores tk1: keys 128-255, queries 128-255 only
                    er1 = er_pool.tile(
                        [128, 256], BF16, tag="er1", name=f"er1_{b}_{j}"
                    )
                    for h2 in range(2):
                        po = 64 * h2
                        sc1 = psA.tile(
                            [128, 128], F32, tag="psA", name=f"sc1_{b}_{j}_{h2}"
                        )
                        nc.tensor.matmul(
                            sc1,
                            kx[po : po + 64, 128:256],
                            qx[po : po + 64, 128:256],
                            start=True,
                            stop=True,
                        )
                        nc.scalar.activation(
                            out=er1[:, h2 * 128 : (h2 + 1) * 128],
                            in_=sc1,
                            func=AF.Exp,
                        )
                    if variant == 3:
                        dbg_consume.append((er0, er1))
                        continue
                    # causal mask on the two diagonal 128-blocks of each
                    # head: keep iff tq >= tk
                    nc.gpsimd.affine_select(
                        out=er0.rearrange("p (h q) -> p h q", q=LS)[:, :, 0:128],
                        in_=er0.rearrange("p (h q) -> p h q", q=LS)[:, :, 0:128],
                        pattern=[[0, 2], [1, 128]],
                        compare_op=OP.is_ge,
                        fill=0.0,
                        base=0,
                        channel_multiplier=-1,
                    )
                    nc.gpsimd.affine_select(
                        out=er1,
                        in_=er1,
                        pattern=[[0, 2], [1, 128]],
                        compare_op=OP.is_ge,
                        fill=0.0,
                        base=0,
                        channel_multiplier=-1,
                    )
                    # per-pair denominator sums [2, LS] (base partition 0)
                    smp = psS.tile([2, LS], F32, tag="psS", name=f"smp_{b}_{j}")
                    for h2 in range(2):
                        nc.tensor.matmul(
                            smp,
                            selh0 if h2 == 0 else selh1,
                            er0[:, h2 * LS : (h2 + 1) * LS],
                            start=(h2 == 0),
                            stop=False,
                            skip_group_check=True,
                        )
                    for h2 in range(2):
                        nc.tensor.matmul(
                            smp[:, 128:256],
                            selh0 if h2 == 0 else selh1,
                            er1[:, h2 * 128 : (h2 + 1) * 128],
                            start=False,
                            stop=(h2 == 1),
                            skip_group_check=True,
                        )
                    # av [64, 512]: head h2 in column half h2
                    av = psA.tile([64, 512], F32, tag="psA", name=f"av_{b}_{j}")
                    for h2 in range(2):
                        head = 2 * j + h2
                        ch, hh = head // 8, head % 8
                        vt0 = v_sb[:, 0, ch * 512 + hh * 64 : ch * 512 + hh * 64 + 64]
                        vt1 = v_sb[:, 1, ch * 512 + hh * 64 : ch * 512 + hh * 64 + 64]
                        nc.tensor.matmul(
                            av[:, h2 * LS : (h2 + 1) * LS],
                            vt0,
                            er0[:, h2 * LS : (h2 + 1) * LS],
                            start=(h2 == 0),
                            stop=False,
                            skip_group_check=True,
                        )
                        nc.tensor.matmul(
                            av[:, h2 * LS + 128 : (h2 + 1) * LS],
                            vt1,
                            er1[:, h2 * 128 : (h2 + 1) * 128],
                            start=False,
                            stop=(h2 == 1),
                            skip_group_check=True,
                        )
                    if variant == 3:
                        continue
                    if variant == 4:
                        dbg_consume.append((av, smp))
                        continue
                    # normalize: rinv = 1/D, broadcast rows via sel2 matmul
                    rinv_f = small.tile(
                        [2, LS], F32, tag="rinvf", name=f"rinvf_{b}_{j}"
                    )
                    nc.vector.reciprocal_approx_fast(out=rinv_f, in_=smp)
                    rinv = small.tile(
                        [2, LS], F32R, tag="rinv", name=f"rinv_{b}_{j}"
                    )
                    nc.vector.tensor_copy(out=rinv, in_=rinv_f)
                    rbp = psS.tile(
                        [128, 256], F32, tag="psS", name=f"rbp_{b}_{j}"
                    )
                    nc.tensor.matmul(
                        rbp,
                        sel2,
                        rinv,
                        start=True,
                        stop=True,
                        skip_group_check=True,
                    )
                    rbp_sb = small.tile(
                        [128, 256], BF16, tag="rbp_sb", name=f"rbps_{b}_{j}"
                    )
                    nc.scalar.activation(out=rbp_sb, in_=rbp, func=AF.Copy)
                    nc.vector.tensor_mul(
                        yT[0:64, j, :], av[:, 0:LS], rbp_sb[0:64, :]
                    )
                    nc.vector.tensor_mul(
                        yT[64:128, j, :], av[:, LS : 2 * LS], rbp_sb[64:128, :]
                    )

                if variant in (3, 4, 5):
                    # consume debug tiles so DCE keeps the chain under test
                    o_ = work.tile([128, C], F32, tag="ostage", name=f"od_{b}")
                    nc.vector.memset(o_, 0.0)
                    if variant == 3:
                        er0d, er1d = dbg_consume[-1]
                        nc.vector.tensor_copy(out=o_[:, 0:512], in_=er0d)
                        nc.vector.tensor_copy(out=o_[:, 512:768], in_=er1d)
                    elif variant == 4:
                        avd, smpd = dbg_consume[-1]
                        nc.vector.tensor_copy(out=o_[0:64, 0:512], in_=avd)
                        nc.vector.tensor_copy(out=o_[0:2, 512:768], in_=smpd[0:2, :])
                    else:
                        nc.vector.tensor_copy(
                            out=o_[:, 0:512],
                            in_=yT[:, 6:8, :].rearrange("p d t -> p (d t)"),
                        )
                    nc.sync.dma_start(out=out[t0 : t0 + 128, :], in_=o_)
                    o1_ = work.tile([128, C], F32, tag="ostage", name=f"od1_{b}")
                    nc.vector.memset(o1_, 0.0)
                    nc.sync.dma_start(out=out[t0 + 128 : t0 + 256, :], in_=o1_)
                    continue

                # ---- proj: out[t, e] = yT.T @ w_proj ----
                for th in range(2):
                    o_ = work.tile([128, C], F32, tag="ostage", name=f"o_{b}_{th}")
                    for ch in range(2):
                        pps = psA.tile(
                            [128, 512], F32, tag="psA", name=f"pps_{b}_{th}_{ch}"
                        )
                        for dt in range(8):
                            nc.tensor.matmul(
                                pps,
                                yT[:, dt, th * 128 : (th + 1) * 128],
                                wp_sb[:, dt, ch * 512 : (ch + 1) * 512],
                                start=(dt == 0),
                                stop=(dt == 7),
                            )
                        if ch == 0:
                            nc.vector.tensor_copy(
                                out=o_[:, ch * 512 : (ch + 1) * 512], in_=pps
                            )
                        else:
                            nc.scalar.activation(
                                out=o_[:, ch * 512 : (ch + 1) * 512],
                                in_=pps,
                                func=AF.Copy,
                            )
                    nc.sync.dma_start(
                        out=out[t0 + th * 128 : t0 + (th + 1) * 128, :], in_=o_
                    )

    nc.finalize()
    return nc


_NC_CACHE = None


def _get_nc():
    global _NC_CACHE
    if _NC_CACHE is None:
        _NC_CACHE = build()
    return _NC_CACHE


_RUNNER_CACHE = None


def _get_runner():
    global _RUNNER_CACHE
    if _RUNNER_CACHE is not None:
        return _RUNNER_CACHE
    import jax
    from jax.sharding import Mesh, PartitionSpec
    from jax.experimental.shard_map import shard_map
    from concourse import bass2jax, mybir as mb

    nc = _get_nc()
    bass2jax.install_neuronx_cc_hook()
    partition_name = nc.partition_id_tensor.name if nc.partition_id_tensor else None
    in_names, out_names, out_avals, zero_shapes = [], [], [], []
    for alloc in nc.m.functions[0].allocations:
        if not isinstance(alloc, mb.MemoryLocationSet):
            continue
        name = alloc.memorylocations[0].name
        if alloc.kind == "ExternalInput":
            if name != partition_name:
                in_names.append(name)
        elif alloc.kind == "ExternalOutput":
            out_names.append(name)
            shape = tuple(alloc.tensor_shape)
            dtype = mb.dt.np(alloc.dtype)
            out_avals.append(jax.core.ShapedArray(shape, dtype))
            zero_shapes.append((shape, dtype))
    n_params = len(in_names)
    all_in = list(in_names) + list(out_names)
    if partition_name is not None:
        all_in.append(partition_name)

    def _body(*args):
        operands = list(args)
        if partition_name is not None:
            operands.append(bass2jax.partition_id_tensor())
        outs = bass2jax._bass_exec_p.bind(
            *operands,
            out_avals=tuple(out_avals),
            in_names=tuple(all_in),
            out_names=tuple(out_names),
            lowering_input_output_aliases=(),
            sim_require_finite=True,
            sim_require_nnan=True,
            nc=nc,
        )
        return tuple(outs)

    devices = jax.devices()[:NCORES]
    mesh = Mesh(np.asarray(devices), ("core",))
    nin = n_params + len(out_names)
    fn = jax.jit(
        shard_map(
            _body,
            mesh=mesh,
            in_specs=(PartitionSpec("core"),) * nin,
            out_specs=(PartitionSpec("core"),) * len(out_names),
            check_rep=False,
        ),
        keep_unused=True,
    )
    _RUNNER_CACHE = (fn, in_names, zero_shapes)
    return _RUNNER_CACHE


def kernel(x, w_qkv, ln_w, w_proj, _trace=False):
    x = np.asarray(x, dtype=np.float32)
    w_qkv = np.asarray(w_qkv, dtype=np.float32)
    ln_w = np.asarray(ln_w, dtype=np.float32)
    w_proj = np.asarray(w_proj, dtype=np.float32)

    x2 = np.ascontiguousarray(x.reshape(B * T, C))
    in_maps = [
        {
            "x": np.ascontiguousarray(x2[i * TOK : (i + 1) * TOK]),
            "w_qkv": w_qkv,
            "ln_w": ln_w,
            "w_proj": w_proj,
        }
        for i in range(NCORES)
    ]
    if _trace:
        nc = _get_nc()
        res = run_bass_kernel_spmd(
            nc, in_maps, core_ids=list(range(NCORES)), trace=True
        )
        full = np.concatenate(
            [res.results[i]["out"] for i in range(NCORES)], axis=0
        )
        return full.reshape(B, T, C).astype(np.float32), res

    fn, in_names, zero_shapes = _get_runner()
    concat_in = [
        np.concatenate([m[name] for m in in_maps], axis=0) for name in in_names
    ]
    concat_zeros = [
        np.zeros((NCORES * shape[0], *shape[1:]), dtype)
        for shape, dtype in zero_shapes
    ]
    outs = fn(*concat_in, *concat_zeros)
    full = np.asarray(outs[0])
    return full.reshape(B, T, C).astype(np.float32)
